# revision 40
# baseline (speedup 1.0000x reference)
"""AttentionGNNLayer on 8 TRN2 NeuronCores (Bass/Tile).

Strategy (src-sharded, collective-free, batched SWDGE):
- Node i belongs to core i // 12500. Each core gets node_emb ROTATED and
  pre-TRANSPOSED on host (embT f32 [128, NPAD]) so its own slice is rows
  [0, 12500) and phase A needs no PE transposes. ER edges assigned to the
  core owning er_src; per-src aggregation is core-local.
- Phase A (per core): PM[n] = emb_rot[n] @ [W1|Wn] (bias-free, stored bf16,
  split into 4 quarter tensors of 25088 rows for int16 gather indices), and
  QS[n] = [emb@W2 + b_attn | emb@Wself + b_self] (f32) for the local slice.
  f32 matmuls (bf16 anywhere in the M0 path costs ~3.6% max-rel error;
  bf16 table STORAGE costs only 0.94%).
- Phase B: edges dealt into 4 quarters (by dst row) x 32 windows x 1024
  slots. Per window: one dma_gather of PM[dst] (512B bf16 rows), one
  dma_gather of Q[src], per-edge logits e = w0 . tanh(P1+Q), g = exp(e+b0),
  one dma_scatter_add of [g*M0 | g] rows (elem 129, stride 192) into U by
  src. SWDGE desc-gen (~8.5ns/descriptor on the Q7) is the main cost: the
  three streams are spread over 4 SWDGE queues which parallelize across Q7
  cores. Scatters alternate over 4 U tables (window % 4) so the 4 RMW
  chains run in parallel and never race (same-src edges never share a
  window within a chain: host deals round-robin over windows after sorting
  by src; max per-quarter degree 12 << 32 windows).
- EE edges reduce to wsum[s] = sum of ee_weight per src (host bincount of
  input scalars); device applies wsum * (M0[s] + bn) in phase C.
- Phase C: out[s] = tanh(SELF + U_num*rr + M0*wsum + bn*(Z*rr + wsum)),
  rr = 1/(Z+1e-9), U summed over the 4 tables.
- No segment max: softmax max-subtraction cancels except the +1e-9 epsilon;
  |e| <= ~6 so exp is well-conditioned in fp32.
"""
import os
import sys

import numpy as np
import ml_dtypes

sys.path.insert(0, "/opt/trn_rl_repo")

import concourse.tile as tile  # noqa: E402
from concourse import bacc, mybir  # noqa: E402
from concourse.bass_utils import run_bass_kernel_spmd  # noqa: E402

dt = mybir.dt
bf16 = dt.bfloat16

N = 100_000
D = 128
NC = 8
SL = N // NC            # 12500 nodes per core
NPAD = 100_352          # 784 blocks of 128
QROWS = NPAD // 4       # 25088 rows per PM quarter (< 32768 for int16)
MEGA = 512              # rows per phase-A megablock
NMEGA_Q = QROWS // MEGA  # 49
LROWS = 13_312          # local-table rows (26 megas; >= 12545)
NLOCM = LROWS // MEGA   # 26
UROWS = 12_672          # U table rows (12544 + trash block)
TRASH = 12_544
UW = 192                # U row stride (f32, 768B)
NW = 32                 # windows per quarter
WSZ = 1024              # edges per window (= idx per SWDGE instruction)
NWT = 4 * NW            # 128 windows total
CMEGA = 7               # phase-C blocks per mega
NCM = 14                # 98 blocks = 14 * 7

_CACHE = {}


def _build():
    if "nc" in _CACHE:
        return _CACHE["nc"]
    nc = bacc.Bacc("TRN2", target_bir_lowering=False, debug=False, num_devices=NC,
                   num_swdge_queues=4)

    embT = nc.dram_tensor("embT", [D, NPAD], dt.float32, kind="ExternalInput")
    wall = nc.dram_tensor("wall", [D, 512], dt.float32, kind="ExternalInput")
    brep = nc.dram_tensor("brep", [128, 256], dt.float32, kind="ExternalInput")
    w0r = nc.dram_tensor("w0r", [128, D], dt.float32, kind="ExternalInput")
    b0c = nc.dram_tensor("b0c", [128, 1], dt.float32, kind="ExternalInput")
    bnr = nc.dram_tensor("bnr", [128, D], dt.float32, kind="ExternalInput")
    eidx = nc.dram_tensor("eidx", [4, 128, NW * 192], dt.int16,
                          kind="ExternalInput")
    wsum = nc.dram_tensor("wsum", [LROWS, 1], dt.float32, kind="ExternalInput")

    out = nc.dram_tensor("out", [TRASH, D], dt.float32, kind="ExternalOutput")

    pm_q = [nc.dram_tensor(f"pm_q{q}", [QROWS, 256], bf16) for q in range(4)]
    qs_d = nc.dram_tensor("qs_d", [LROWS, 256], dt.float32)
    u_ds = [nc.dram_tensor(f"u_d{k}", [UROWS, UW], dt.float32) for k in range(4)]

    with tile.TileContext(nc) as tc:
        with tc.tile_pool(name="const", bufs=1) as cpool:
            wall_t = cpool.tile([128, 512], dt.float32)
            nc.sync.dma_start(out=wall_t[:], in_=wall[:, :])
            brep_t = cpool.tile([128, 256], dt.float32)
            nc.sync.dma_start(out=brep_t[:], in_=brep[:, :])
            w0_t = cpool.tile([128, D], dt.float32)
            nc.sync.dma_start(out=w0_t[:], in_=w0r[:, :])
            b0_t = cpool.tile([128, 1], dt.float32)
            nc.sync.dma_start(out=b0_t[:], in_=b0c[:, :])
            bn_t = cpool.tile([128, D], dt.float32)
            nc.sync.dma_start(out=bn_t[:], in_=bnr[:, :])

            # zero the U tables: 99 blocks in chunks of 11
            zt = cpool.tile([128, 11 * UW], dt.float32)
            nc.vector.memset(zt[:], 0.0)
            with nc.named_scope("init"):
                for u_d in u_ds:
                    for c in range(9):
                        r0 = c * 11 * 128
                        nc.sync.dma_start(
                            out=u_d.ap()[r0:r0 + 11 * 128, :].rearrange(
                                "(a p) c -> p a c", p=128),
                            in_=zt[:].rearrange("p (a c) -> p a c", c=UW))

            with tc.tile_pool(name="pa", bufs=4) as pa, \
                 tc.tile_pool(name="paps", bufs=6, space="PSUM") as paps, \
                 tc.tile_pool(name="pbi", bufs=2) as pbi, \
                 tc.tile_pool(name="pb", bufs=8) as pb:
                for qq in range(4):
                    with nc.named_scope(f"phaseA{qq}"):
                        for m in range(NMEGA_Q):
                            gm = qq * NMEGA_Q + m
                            r0 = gm * MEGA
                            ebT = pa.tile([128, MEGA], dt.float32, tag="ebT")
                            nc.sync.dma_start(
                                out=ebT[:], in_=embT.ap()[:, r0:r0 + MEGA])
                            pm = pa.tile([128, 4, 256], bf16, tag="pm")
                            local = gm < NLOCM
                            w = 512 if local else 256
                            if local:
                                qs = pa.tile([128, 4, 256], dt.float32, tag="qs")
                            for j in range(4):
                                ps = paps.tile([128, 512], dt.float32, tag="ps")
                                nc.tensor.matmul(
                                    out=ps[:, 0:w],
                                    lhsT=ebT[:, j * 128:(j + 1) * 128],
                                    rhs=wall_t[:, 0:w], start=True, stop=True)
                                nc.vector.tensor_copy(out=pm[:, j, :],
                                                      in_=ps[:, 0:256])
                                if local:
                                    nc.vector.tensor_add(out=qs[:, j, :],
                                                         in0=ps[:, 256:512],
                                                         in1=brep_t[:])
                            nc.scalar.dma_start(
                                out=pm_q[qq].ap()[m * MEGA:(m + 1) * MEGA, :]
                                    .rearrange("(a p) c -> p a c", p=128),
                                in_=pm[:])
                            if local:
                                nc.scalar.dma_start(
                                    out=qs_d.ap()[r0:r0 + MEGA, :]
                                        .rearrange("(a p) c -> p a c", p=128),
                                    in_=qs[:])

                    with nc.named_scope(f"phaseB{qq}"):
                        # all 32 windows' indices for this quarter in one DMA
                        qit = pbi.tile([128, NW, 192], dt.int16, tag="qit")
                        nc.sync.dma_start(
                            out=qit[:],
                            in_=eidx.ap()[qq].rearrange(
                                "p (w c) -> p w c", c=192))

                        def emit_gather(w):
                            wi = qq * NW + w
                            pmt = pb.tile([128, 8 * 256], bf16, tag="pmt")
                            nc.gpsimd.dma_gather(
                                out_ap=pmt[:].rearrange("p (g c) -> p g c", c=256),
                                in_ap=pm_q[qq].ap()[:, :],
                                idxs_ap=qit[:, w, 0:64],
                                num_idxs=WSZ, num_idxs_reg=WSZ, elem_size=256,
                                queue_num=wi % 4)
                            qt = pb.tile([128, 8 * 128], dt.float32, tag="qt")
                            nc.gpsimd.dma_gather(
                                out_ap=qt[:].rearrange("p (g c) -> p g c", c=128),
                                in_ap=qs_d.ap()[:, 0:128],
                                idxs_ap=qit[:, w, 64:128],
                                num_idxs=WSZ, num_idxs_reg=WSZ,
                                elem_size=128, elem_step=256,
                                queue_num=(wi + 1) % 4)
                            return pmt, qt

                        def emit_compute(w, pmt, qt):
                            wi = qq * NW + w
                            pm3 = pmt[:].rearrange("p (g c) -> p g c", c=256)
                            q3 = qt[:].rearrange("p (g c) -> p g c", c=128)
                            nc.vector.tensor_add(out=q3, in0=q3,
                                                 in1=pm3[:, :, 0:128])
                            nc.scalar.activation(
                                out=q3, in_=q3,
                                func=mybir.ActivationFunctionType.Tanh)
                            w03 = w0_t[:].rearrange("p (o c) -> p o c", o=1) \
                                .to_broadcast([128, 8, 128])
                            nc.vector.tensor_mul(out=q3, in0=q3, in1=w03)
                            et = pb.tile([128, 8], dt.float32, tag="et")
                            nc.vector.reduce_sum(out=et[:], in_=q3,
                                                 axis=mybir.AxisListType.X)
                            gt = pb.tile([128, 8], dt.float32, tag="gt")
                            nc.scalar.activation(
                                out=gt[:], in_=et[:],
                                func=mybir.ActivationFunctionType.Exp,
                                bias=b0_t[:])
                            uin = pb.tile([128, 8 * 129], dt.float32, tag="uin")
                            u3 = uin[:].rearrange("p (g c) -> p g c", c=129)
                            g3 = gt[:].rearrange("p (g o) -> p g o", o=1) \
                                .to_broadcast([128, 8, 128])
                            nc.vector.tensor_mul(out=u3[:, :, 0:128],
                                                 in0=pm3[:, :, 128:256], in1=g3)
                            nc.vector.tensor_copy(
                                out=u3[:, :, 128:129],
                                in_=gt[:].rearrange("p (g o) -> p g o", o=1))
                            nc.gpsimd.dma_scatter_add(
                                out_ap=u_ds[wi % 4].ap()[:, 0:129],
                                in_ap=u3,
                                idxs_ap=qit[:, w, 128:192],
                                num_idxs=WSZ, num_idxs_reg=WSZ,
                                elem_size=129, elem_step=UW,
                                queue_num=(wi + 2) % 4)

                        # software pipeline: gathers run LAG windows ahead of
                        # compute+scatter so scatters don't stall the in-order
                        # Pool queue behind the DVE chain.
                        LAG = 2
                        pend = []
                        for w in range(NW):
                            pend.append((w, *emit_gather(w)))
                            if len(pend) > LAG:
                                emit_compute(*pend.pop(0))
                        for item in pend:
                            emit_compute(*item)

            with nc.named_scope("phaseC"), tc.tile_pool(name="pc", bufs=3) as pc:
                for m in range(NCM):
                    r0 = m * CMEGA * 128
                    nr = CMEGA * 128
                    uts = []
                    for k in range(4):
                        utk = pc.tile([128, CMEGA, 130], dt.float32, tag=f"ut{k}")
                        nc.sync.dma_start(
                            out=utk[:],
                            in_=u_ds[k].ap()[r0:r0 + nr, 0:130]
                                .rearrange("(a p) c -> p a c", p=128))
                        uts.append(utk)
                    ut = uts[0]
                    nc.vector.tensor_add(out=ut[:], in0=ut[:], in1=uts[1][:])
                    nc.vector.tensor_add(out=uts[2][:], in0=uts[2][:],
                                         in1=uts[3][:])
                    nc.vector.tensor_add(out=ut[:], in0=ut[:], in1=uts[2][:])
                    m0 = pc.tile([128, CMEGA, 128], bf16, tag="m0")
                    nc.sync.dma_start(
                        out=m0[:],
                        in_=pm_q[0].ap()[r0:r0 + nr, 128:256]
                            .rearrange("(a p) c -> p a c", p=128))
                    sf = pc.tile([128, CMEGA, 128], dt.float32, tag="sf")
                    nc.sync.dma_start(
                        out=sf[:],
                        in_=qs_d.ap()[r0:r0 + nr, 128:256]
                            .rearrange("(a p) c -> p a c", p=128))
                    ws = pc.tile([128, CMEGA, 1], dt.float32, tag="ws")
                    nc.sync.dma_start(
                        out=ws[:],
                        in_=wsum.ap()[r0:r0 + nr, :]
                            .rearrange("(a p) c -> p a c", p=128))
                    ob = pc.tile([128, CMEGA, 128], dt.float32, tag="ob")
                    for b in range(CMEGA):
                        zz = ut[:, b, 128:129]
                        zp = pc.tile([128, 1], dt.float32, tag="zp")
                        nc.vector.tensor_scalar_add(zp[:], zz, 1e-9)
                        rr = pc.tile([128, 1], dt.float32, tag="rr")
                        nc.vector.reciprocal(rr[:], zp[:])
                        # zr = Z*rr + wsum
                        zr = pc.tile([128, 1], dt.float32, tag="zr")
                        nc.vector.scalar_tensor_tensor(
                            out=zr[:], in0=zz, scalar=rr[:], in1=ws[:, b, :],
                            op0=mybir.AluOpType.mult, op1=mybir.AluOpType.add)
                        # t1 = M0 * wsum
                        t1 = pc.tile([128, 128], dt.float32, tag="t1")
                        nc.vector.tensor_scalar_mul(t1[:], m0[:, b, :],
                                                    ws[:, b, :])
                        # s1 = U_num*rr + t1
                        s1 = pc.tile([128, 128], dt.float32, tag="s1")
                        nc.vector.scalar_tensor_tensor(
                            out=s1[:], in0=ut[:, b, 0:128], scalar=rr[:],
                            in1=t1[:], op0=mybir.AluOpType.mult,
                            op1=mybir.AluOpType.add)
                        # s1 += bn*zr
                        nc.vector.scalar_tensor_tensor(
                            out=s1[:], in0=bn_t[:], scalar=zr[:], in1=s1[:],
                            op0=mybir.AluOpType.mult, op1=mybir.AluOpType.add)
                        nc.vector.tensor_add(out=s1[:], in0=s1[:],
                                             in1=sf[:, b, :])
                        nc.scalar.activation(
                            out=ob[:, b, :], in_=s1[:],
                            func=mybir.ActivationFunctionType.Tanh)
                    nc.sync.dma_start(
                        out=out.ap()[r0:r0 + nr, :]
                            .rearrange("(a p) c -> p a c", p=128),
                        in_=ob[:])

    nc.compile()
    _CACHE["nc"] = nc
    return nc


def _idx_layout(arr):
    """[NWT, WSZ] int -> [NWT, 128, 64] int16 SWDGE idx layout: idx i of a
    window at [16r + i%16, i//16] for r in 0..7 (replicated over partition
    groups; the Q7 reads channels from partitions 16..31)."""
    nwt = arr.shape[0]
    a = arr.reshape(nwt, WSZ // 16, 16).transpose(0, 2, 1)  # [NWT, 16, 64]
    return np.tile(a, (1, 8, 1)).astype(np.int16)


def kernel(node_emb, er_src, er_dst, ee_src, ee_dst, ee_weight,
           W_attn_w, W_attn_b, w0_w, w0_b, W_self_w, W_self_b,
           W_neigh_w, W_neigh_b, **_):
    node_emb = np.asarray(node_emb, np.float32)
    er_src = np.asarray(er_src).astype(np.int64)
    er_dst = np.asarray(er_dst).astype(np.int64)
    ee_src = np.asarray(ee_src).astype(np.int64)
    ee_weight = np.asarray(ee_weight, np.float32)

    wall = np.concatenate([
        np.asarray(W_attn_w, np.float32)[:D],        # W1 (dst side)
        np.asarray(W_neigh_w, np.float32),           # Wn (NO bias)
        np.asarray(W_attn_w, np.float32)[D:],        # W2 (src side)
        np.asarray(W_self_w, np.float32)], axis=1)
    brep = np.broadcast_to(
        np.concatenate([np.asarray(W_attn_b, np.float32),
                        np.asarray(W_self_b, np.float32)]),
        (128, 256)).copy()
    w0rep = np.broadcast_to(np.asarray(w0_w, np.float32), (128, D)).copy()
    b0c = np.full((128, 1), float(np.asarray(w0_b)), np.float32)
    bnr = np.broadcast_to(np.asarray(W_neigh_b, np.float32), (128, D)).copy()

    embT_full = np.ascontiguousarray(node_emb.T)  # [D, N] f32

    in_maps = []
    for c in range(NC):
        lo = c * SL
        embT_rot = np.concatenate([
            np.roll(embT_full, -lo, axis=1),
            np.zeros((D, NPAD - N), np.float32)], axis=1)

        sel = (er_src >= lo) & (er_src < lo + SL)
        s_all = (er_src[sel] - lo).astype(np.int32)
        d_all = ((er_dst[sel] - lo) % N).astype(np.int32)
        qq_all = d_all // QROWS

        pm_i = np.zeros((NWT, WSZ), np.int32)
        q_i = np.full((NWT, WSZ), TRASH, np.int32)
        s_i = np.full((NWT, WSZ), TRASH, np.int32)
        for q in range(4):
            msk = qq_all == q
            sq = s_all[msk]
            dq = d_all[msk] - q * QROWS
            o = np.argsort(sq, kind="stable")
            sq, dq = sq[o], dq[o]
            nq = len(sq)
            assert nq <= NW * WSZ, (c, q, nq)
            win = q * NW + np.arange(nq) % NW
            slot = np.arange(nq) // NW
            pm_i[win, slot] = dq
            q_i[win, slot] = sq
            s_i[win, slot] = sq

        eidx = np.concatenate(
            [_idx_layout(pm_i), _idx_layout(q_i), _idx_layout(s_i)], axis=2)
        # [NWT, 128, 192] -> [4 quarters, 128 partitions, NW*192]
        eidx = np.ascontiguousarray(
            eidx.reshape(4, NW, 128, 192).transpose(0, 2, 1, 3)
        ).reshape(4, 128, NW * 192)

        esel = (ee_src >= lo) & (ee_src < lo + SL)
        wsum = np.bincount(ee_src[esel] - lo, weights=ee_weight[esel],
                           minlength=LROWS)[:LROWS].astype(np.float32)

        in_maps.append({
            "embT": embT_rot, "wall": wall, "brep": brep, "w0r": w0rep,
            "b0c": b0c, "bnr": bnr, "eidx": eidx,
            "wsum": wsum.reshape(LROWS, 1),
        })

    nc = _build()
    trace = os.environ.get("BASS_KERNEL_TRACE") == "1"
    res = run_bass_kernel_spmd(nc, in_maps, core_ids=list(range(NC)), trace=trace)
    _CACHE["last_res"] = res
    return np.concatenate([res.results[c]["out"][:SL] for c in range(NC)], axis=0)
